# revision 37
# baseline (speedup 1.0000x reference)
"""CRsAE1d FISTA kernel for 8 Trainium2 NeuronCores.

Strategy
--------
H = [circ(f_0)|...|circ(f_7)] is block-circulant: with 128-row blocking each
circulant is block-bidiagonal with ONE repeated diagonal block D_k (lower-band
Toeplitz, f[0..63]) and ONE repeated subdiagonal block S_k (upper-corner band,
f[1..63]).  So H@w and H^T@v are 16 tiny [128,128] matmuls each (per filter:
one D-band + one S-band matmul whose rhs is a block-shifted view of the
operand, with circular wrap handled by an 8-column halo copy).

Data-parallel over batch: 64 columns -> 8 cores x 8 columns.  Everything lives
in SBUF; per iteration the PE does 48 fp16 matmuls (conv1 16, conv2 16,
momentum-passthrough 16 via +/- m_t scaled-identity weights accumulated into
the same PSUM region), DVE does the v-combine and softshrink
(x - clamp(x, -thr, thr)) with the f32 master copy of x kept exactly.

fp16 weights/activations with an f32 master for x gives ~4e-4 rel err vs the
f32 reference (measured in numpy bit-sim); bf16 would give ~3e-3.
"""

import sys

for p in ("/root/.axon_site", "/root/.axon_site/_ro/trn_rl_repo",
          "/root/.axon_site/_ro/pypackages", "/opt/trn_rl_repo"):
    if p not in sys.path:
        sys.path.append(p)

import numpy as np

T = 15
LAM = 0.1
N = 2048
K = 8
KS = 64
B = 64
NCORES = 8
BL = B // NCORES          # batch per core
NB = N // 128             # 16 row-blocks
CW = NB * BL              # 128 columns per (k) region in (J,b) layout
CWH = CW + BL             # body + 8-column halo

_CACHE: dict = {}


def _momentum_coeffs():
    s = 0.0
    ms = []
    for _ in range(T):
        st = (1.0 + np.sqrt(1.0 + 4.0 * s * s)) / 2.0
        ms.append(np.float32((s - 1.0) / st))
        s = st
    return ms


def _band_matrices(D):
    """D_k[r,s] = f_k[r-s] for 0<=r-s<KS;  S_k[r,s] = f_k[128+r-s] for s-r>=65."""
    Dm = np.zeros((K, 128, 128), np.float32)
    Sm = np.zeros((K, 128, 128), np.float32)
    r = np.arange(128)[:, None]
    s = np.arange(128)[None, :]
    d1 = r - s
    d2 = 128 + r - s
    m1 = (d1 >= 0) & (d1 < KS)
    m2 = (d2 > 0) & (d2 < KS)
    for k in range(K):
        Dm[k][m1] = D[k][d1[m1]]
        Sm[k][m2] = D[k][d2[m2]]
    return Dm, Sm


def _dedup_ldweights(d):
    """Remove Ldweights whose weight AP is identical to the previous PE
    weight load with only Matmults in between — the stationary operand is
    still in the array.  (bass emits one Ldweights per matmul, even for
    back-to-back matmuls sharing lhsT.)  Any waits on a removed Ldweights
    move onto the next PE instruction (the legalizer splits them later)."""
    for fn in d["functions"]:
        for bb in fn["blocks"]:
            out = []
            prev_key = None
            pending_waits = []
            for inst in bb["instructions"]:
                op = inst["opcode"]
                if op == "Ldweights":
                    w = inst["ins"][0]
                    key = (w.get("memref"), w.get("offset"), str(w.get("ap")),
                           str(inst.get("tile_position")))
                    si = inst.get("sync_info")
                    if key == prev_key:
                        if si and si.get("on_wait"):
                            pending_waits.extend(si["on_wait"])
                        assert not (si and si.get("on_update"))
                        continue
                    prev_key = key
                elif op == "Matmult":
                    if pending_waits:
                        si = inst.get("sync_info")
                        if si is None:
                            si = {"on_wait": [], "on_update": []}
                            inst["sync_info"] = si
                        si["on_wait"] = list(si.get("on_wait", [])) + pending_waits
                        pending_waits = []
                elif inst.get("engine") == "PE":
                    prev_key = None
                    if pending_waits:
                        si = inst.get("sync_info")
                        if si is None:
                            si = {"on_wait": [], "on_update": []}
                            inst["sync_info"] = si
                        si["on_wait"] = list(si.get("on_wait", [])) + pending_waits
                        pending_waits = []
                out.append(inst)
            assert not pending_waits
            bb["instructions"] = out
    return d


def _legalize_bir(bir_bytes):
    """The walrus build here encodes at most ONE sync-wait per instruction
    ("Too many sync wait commands").  Tile attaches up to 3.  Split the
    extras onto EventSemaphore wait-carrier instructions inserted just
    before, on the same engine (engine streams keep BB relative order, so
    the carriers execute immediately before the original)."""
    import orjson

    d = orjson.loads(bir_bytes)
    _dedup_ldweights(d)
    for fn in d["functions"]:
        for bb in fn["blocks"]:
            out = []
            for inst in bb["instructions"]:
                si = inst.get("sync_info")
                ow = si.get("on_wait", []) if si else []
                if len(ow) > 1:
                    for j, w in enumerate(ow[:-1]):
                        out.append({
                            "debug": inst.get("debug", 0),
                            "engine": inst["engine"],
                            "ins": [],
                            "outs": [],
                            "name": f"{inst['name']}_wsplit{j}",
                            "opcode": "EventSemaphore",
                            "sync_info": {"on_update": [], "on_wait": [w]},
                        })
                    si["on_wait"] = [ow[-1]]
                out.append(inst)
            bb["instructions"] = out
    return orjson.dumps(d)


def _install_patches():
    import concourse.bass2jax as b2j
    from concourse.bass_utils import compile_bir_kernel as _cbk

    def _cbk_legal(bir_str, compile_dir_path, neff_name):
        return _cbk(_legalize_bir(bir_str), compile_dir_path,
                    neff_name=neff_name)

    b2j.compile_bir_kernel = _cbk_legal


def _build_program():
    import concourse.bass as bass
    import concourse.mybir as mybir
    import concourse.tile as tile
    import bass_rust
    from concourse.tile import add_dep_helper as add_dep
    from concourse.vector_clock import ScopedClock

    _install_patches()

    # The nix walrus build rejects >1 sync-wait on CTRL-class (Drain)
    # instructions; split the Tile tail-drain waits across a chain of
    # single-wait drains.
    def _drain_and_barrier(self, tick_clock, wait_clock):
        drain_inst = self.nc.sync.drain()
        wait_clock.add_sem_waits(
            drain_inst.ins, ScopedClock({None: tick_clock.global_clock})
        )
        si = drain_inst.ins.sync_info
        waits = list(si.on_wait) if si is not None else []
        if len(waits) > 1:
            si.on_wait = waits[:1]
            for w in waits[1:]:
                d = self.nc.sync.drain()
                d.ins.sync_info = bass_rust.SyncInfo(on_wait=[w], on_update=[])
        self.nc.all_engine_barrier()
        assert self.sems is not None
        popped = self.nc._tile_sem_poison_stack.pop()
        assert popped is self._sem_poison
        self.nc.clear_and_free_semaphores(list(self.sems.allocated().values()))
        self.nc.all_engine_barrier()

    tile.TileContext._drain_and_barrier = _drain_and_barrier

    f32 = mybir.dt.float32
    f16 = mybir.dt.float16
    Alu = mybir.AluOpType
    ms = _momentum_coeffs()

    nc = bass.Bass("TRN2", target_bir_lowering=False, debug=False,
                   num_devices=NCORES)
    d_sig = nc.dram_tensor("sig", [128, CW], f32, kind="ExternalInput").ap()
    d_w1 = nc.dram_tensor("w1", [128, 2 * K * 128], f16, kind="ExternalInput").ap()
    d_w2 = nc.dram_tensor("w2", [128, 2 * K * 128], f16, kind="ExternalInput").ap()
    d_wid = nc.dram_tensor("wid", [128, 2 * T * 128], f16, kind="ExternalInput").ap()
    d_out = nc.dram_tensor("xout", [128, K * CW], f32, kind="ExternalOutput").ap()

    with tile.TileContext(nc) as tc:
        with (
            tc.tile_pool(name="const", bufs=1) as const,
            tc.tile_pool(name="state", bufs=1) as state,
            tc.tile_pool(name="psq", bufs=2, space="PSUM") as psqp,
            tc.tile_pool(name="psu", bufs=2, space="PSUM") as psup,
            tc.tile_pool(name="psj", bufs=1, space="PSUM") as psjp,
        ):
            w1 = const.tile([128, 2 * K * 128], f16)
            w2 = const.tile([128, 2 * K * 128], f16)
            wid = const.tile([128, 2 * T * 128], f16)
            sigt = const.tile([128, CW], f32)
            nc.sync.dma_start(sigt[:], d_sig[:])
            nc.sync.dma_start(w2[:], d_w2[:])   # t=0 needs w2 first
            nc.sync.dma_start(w1[:], d_w1[:])
            nc.sync.dma_start(wid[:], d_wid[:])

            # All iteration state is split into two independent half-tensors
            # (k 0-3 / k 4-7, one PSUM bank each) so Tile's per-tile
            # dependency tracking lets iteration t+1's conv1 start as soon
            # as half 0 has been shrunk+cast, while half 1 is still in
            # flight.
            KH = K // 2               # 4 filters per half
            HB = KH * CW              # 512 columns per half
            X32h = [state.tile([128, HB], f32, name=f"X32h{h}")
                    for h in range(2)]
            Xbh = [[state.tile([128, KH * CWH], f16, name=f"Xb{a}{h}")
                    for h in range(2)]
                   for a in range(2)]             # [buf A/B][half]
            v16 = state.tile([128, CWH], f16)
            btA = state.tile([128, CW], f32)
            btB = state.tile([128, CW], f32)
            u_h = [state.tile([128, HB], f32, name=f"u_h{h}")
                   for h in range(2)]
            tclh = [state.tile([128, HB], f32, name=f"tclh{h}")
                    for h in range(2)]

            for h in range(2):
                nc.gpsimd.memset(X32h[h][:], 0.0)
                nc.gpsimd.memset(Xbh[0][h][:], 0.0)
                nc.gpsimd.memset(Xbh[1][h][:], 0.0)

            X32h3 = [x.rearrange("p (k c) -> p k c", k=KH) for x in X32h]

            def body(thr_f: float):
                for t in range(T):
                    m = float(ms[t])
                    Xc, Xp = (Xbh[0], Xbh[1]) if t % 2 == 0 else (Xbh[1], Xbh[0])
                    # btmp_t = m_t * r_{t-1} + sig; r_0 = 0 so t<=1 uses sig
                    bt_cur = sigt if t <= 1 else (btA if t % 2 == 0 else btB)
                    bt_next = btB if t % 2 == 0 else btA

                    psu = [psup.tile([128, HB], f32, tag=f"psu{h}",
                                     name=f"psu{h}_{t}")
                           for h in range(2)]

                    if t > 0:
                        # conv1: q = H @ x_t (accumulate over k, 2 bands).
                        # Emit k 0-3 (half 0), then half 0's momentum
                        # matmuls, then k 4-7 — half 1's fp16 copy lands
                        # later than half 0's, so give the PE half-0-only
                        # work to chew on in between.
                        psq = psqp.tile([128, CW], f32)
                        for k in range(K):
                            if k == KH and t > 0:
                                for km in range(KH):
                                    nc.tensor.matmul(
                                        psu[0][:, km * CW:(km + 1) * CW],
                                        wid[:, (2 * t) * 128:(2 * t + 1) * 128],
                                        Xc[0][:, km * CWH + BL:(km + 1) * CWH],
                                        start=(km == 0), stop=False)
                                for km in range(KH):
                                    nc.tensor.matmul(
                                        psu[0][:, km * CW:(km + 1) * CW],
                                        wid[:, (2 * t + 1) * 128:(2 * t + 2) * 128],
                                        Xp[0][:, km * CWH + BL:(km + 1) * CWH],
                                        start=False, stop=False)
                            Xs = Xc[k // KH]
                            o = (k % KH) * CWH
                            nc.tensor.matmul(
                                psq[:],
                                w1[:, (2 * k) * 128:(2 * k + 1) * 128],
                                Xs[:, o + BL: o + CWH],
                                start=(k == 0), stop=False,
                            )
                            nc.tensor.matmul(
                                psq[:],
                                w1[:, (2 * k + 1) * 128:(2 * k + 2) * 128],
                                Xs[:, o: o + CW],
                                start=False, stop=(k == K - 1),
                            )

                        # v = -(1+m) q + btmp (written directly as fp16);
                        # btmp_{t+1} = m_{t+1} q + sig
                        nc.vector.scalar_tensor_tensor(
                            v16[:, 0:CW], psq[:], -(1.0 + m), bt_cur[:],
                            Alu.mult, Alu.add)
                        nc.vector.tensor_copy(v16[:, CW:CWH], v16[:, 0:BL])
                        if t + 1 < T:
                            nc.vector.scalar_tensor_tensor(
                                bt_next[:], psq[:], float(ms[t + 1]), sigt[:],
                                Alu.mult, Alu.add)
                    else:
                        # t=0: x=0 -> q=0, v = sig; momentum terms all zero
                        nc.scalar.copy(v16[:, 0:CW], sigt[:])
                        nc.vector.tensor_copy(v16[:, CW:CWH], v16[:, 0:BL])

                    # Per half: momentum passthrough (no v dependency — runs
                    # while DVE does the v-chain), then conv2 bands, then the
                    # shrink chain.  Emitting h=0's PSUM work before h=1's
                    # lets the h=0 shrink overlap h=1's matmuls.
                    h0_tail = None
                    ts_h = [None, None]
                    tt_h = [None, None]
                    for h in range(2):
                        if t > 0 and h == 1:
                            for k in range(KH):
                                nc.tensor.matmul(
                                    psu[h][:, k * CW:(k + 1) * CW],
                                    wid[:, (2 * t) * 128:(2 * t + 1) * 128],
                                    Xc[h][:, k * CWH + BL:(k + 1) * CWH],
                                    start=(k == 0), stop=False)
                            for k in range(KH):
                                nc.tensor.matmul(
                                    psu[h][:, k * CW:(k + 1) * CW],
                                    wid[:, (2 * t + 1) * 128:(2 * t + 2) * 128],
                                    Xp[h][:, k * CWH + BL:(k + 1) * CWH],
                                    start=False, stop=False)
                        for k in range(KH):
                            kg = h * KH + k
                            reg = psu[h][:, k * CW:(k + 1) * CW]
                            nc.tensor.matmul(
                                reg, w2[:, (2 * kg) * 128:(2 * kg + 1) * 128],
                                v16[:, 0:CW],
                                start=(t == 0 and k == 0), stop=False)
                            nc.tensor.matmul(
                                reg, w2[:, (2 * kg + 1) * 128:(2 * kg + 2) * 128],
                                v16[:, BL:CWH], start=False,
                                stop=(k == KH - 1))

                        # softshrink x_{t+1} = u - clamp(u,-thr,thr), u=x+psu
                        Xp3 = Xp[h].rearrange("p (k c) -> p k c", k=KH)
                        i_stt = nc.vector.scalar_tensor_tensor(
                            u_h[h][:], psu[h][:], 1.0, X32h[h][:],
                            Alu.mult, Alu.add)
                        del i_stt  # scheduler interleaving is net-positive
                        i_ts = nc.vector.tensor_scalar(
                            tclh[h][:], u_h[h][:], -thr_f, thr_f,
                            Alu.max, Alu.min)
                        ts_h[h] = i_ts
                        i_tt = nc.vector.tensor_sub(
                            X32h[h][:], u_h[h][:], tclh[h][:])
                        tt_h[h] = i_tt
                        # fp16 copies on DVE right behind the sub — no
                        # cross-engine hop on the path to t+1's conv1
                        nc.vector.tensor_copy(
                            Xp3[:, :, BL:CWH], X32h3[h][:])
                        nc.vector.tensor_copy(
                            Xp3[:, :, 0:BL],
                            X32h3[h][:, :, CW - BL:CW])

                    if t > 0:
                        # HAM keep-warm: throwaway matmuls pinned to run
                        # mid-shrink (after each half's clamp) so the PE
                        # never sees a full 3.4us idle window and stays at
                        # 2.4GHz.
                        for j, anchor in enumerate(
                                (ts_h[0], ts_h[1], tt_h[1])):
                            junk = psjp.tile([128, CW], f32,
                                             name=f"junk_{t}_{j}", tag="junk")
                            i_j = nc.tensor.matmul(
                                junk[:], w2[:, 0:128], v16[:, 0:CW],
                                start=True, stop=True)
                            add_dep(i_j.ins, anchor.ins, sync=True,
                                    reason="HAM keep-warm mid-gap")

            body(_CACHE["thr"])
            nc.sync.dma_start(d_out[:, 0:HB], X32h[0][:])
            nc.sync.dma_start(d_out[:, HB:2 * HB], X32h[1][:])

    return nc


def kernel(signal, local_dictionary):
    sig = np.ascontiguousarray(np.asarray(signal, dtype=np.float32))
    D = np.ascontiguousarray(np.asarray(local_dictionary, dtype=np.float32))
    assert sig.shape == (N, B) and D.shape == (K, KS)

    # Lipschitz constant: H H^T = F^H diag(sum_k |fft(f_k)|^2) F  (circulants)
    fpad = np.zeros((K, N), np.float64)
    fpad[:, :KS] = D.astype(np.float64)
    L = np.float32((np.abs(np.fft.fft(fpad, axis=1)) ** 2).sum(0).max() + 1.0)
    thr = np.float32(LAM / L)
    _CACHE["thr"] = float(thr)

    Dm, Sm = _band_matrices(D)
    ms = _momentum_coeffs()

    # conv1 lhsT[j,i] = D_k[i,j]  (transposed);  conv2 lhsT[i,j] = D_k[i,j]/L
    w1 = np.empty((128, 2 * K * 128), np.float16)
    w2 = np.empty((128, 2 * K * 128), np.float16)
    for k in range(K):
        w1[:, (2 * k) * 128:(2 * k + 1) * 128] = Dm[k].T.astype(np.float16)
        w1[:, (2 * k + 1) * 128:(2 * k + 2) * 128] = Sm[k].T.astype(np.float16)
        w2[:, (2 * k) * 128:(2 * k + 1) * 128] = (Dm[k] / L).astype(np.float16)
        w2[:, (2 * k + 1) * 128:(2 * k + 2) * 128] = (Sm[k] / L).astype(np.float16)
    eye = np.eye(128, dtype=np.float32)
    wid = np.empty((128, 2 * T * 128), np.float16)
    for t in range(T):
        wid[:, (2 * t) * 128:(2 * t + 1) * 128] = (ms[t] * eye).astype(np.float16)
        wid[:, (2 * t + 1) * 128:(2 * t + 2) * 128] = (-ms[t] * eye).astype(np.float16)

    nc = _build_program()

    from concourse.bass_utils import run_bass_kernel_spmd

    in_maps = []
    for c in range(NCORES):
        sc = sig[:, c * BL:(c + 1) * BL]                      # [2048, 8]
        sc = sc.reshape(NB, 128, BL).transpose(1, 0, 2).reshape(128, CW)
        in_maps.append({
            "sig": np.ascontiguousarray(sc),
            "w1": w1, "w2": w2, "wid": wid,
        })

    _CACHE["in_maps"] = in_maps
    res = run_bass_kernel_spmd(nc, in_maps, list(range(NCORES)))

    out = np.empty((K * N, B), np.float32)
    for c in range(NCORES):
        xc = res.results[c]["xout"]                           # [128, 1024]
        xc = xc.reshape(128, K, NB, BL).transpose(1, 2, 0, 3).reshape(K * N, BL)
        out[:, c * BL:(c + 1) * BL] = xc
    return out


# revision 38
# speedup vs baseline: 1.0040x; 1.0040x over previous
"""CRsAE1d FISTA kernel for 8 Trainium2 NeuronCores.

Strategy
--------
H = [circ(f_0)|...|circ(f_7)] is block-circulant: with 128-row blocking each
circulant is block-bidiagonal with ONE repeated diagonal block D_k (lower-band
Toeplitz, f[0..63]) and ONE repeated subdiagonal block S_k (upper-corner band,
f[1..63]).  So H@w and H^T@v are 16 tiny [128,128] matmuls each (per filter:
one D-band + one S-band matmul whose rhs is a block-shifted view of the
operand, with circular wrap handled by an 8-column halo copy).

Data-parallel over batch: 64 columns -> 8 cores x 8 columns.  Everything lives
in SBUF; per iteration the PE does 48 fp16 matmuls (conv1 16, conv2 16,
momentum-passthrough 16 via +/- m_t scaled-identity weights accumulated into
the same PSUM region), DVE does the v-combine and softshrink
(x - clamp(x, -thr, thr)) with the f32 master copy of x kept exactly.

fp16 weights/activations with an f32 master for x gives ~4e-4 rel err vs the
f32 reference (measured in numpy bit-sim); bf16 would give ~3e-3.
"""

import sys

for p in ("/root/.axon_site", "/root/.axon_site/_ro/trn_rl_repo",
          "/root/.axon_site/_ro/pypackages", "/opt/trn_rl_repo"):
    if p not in sys.path:
        sys.path.append(p)

import numpy as np

T = 15
LAM = 0.1
N = 2048
K = 8
KS = 64
B = 64
NCORES = 8
BL = B // NCORES          # batch per core
NB = N // 128             # 16 row-blocks
CW = NB * BL              # 128 columns per (k) region in (J,b) layout
CWH = CW + BL             # body + 8-column halo

_CACHE: dict = {}


def _momentum_coeffs():
    s = 0.0
    ms = []
    for _ in range(T):
        st = (1.0 + np.sqrt(1.0 + 4.0 * s * s)) / 2.0
        ms.append(np.float32((s - 1.0) / st))
        s = st
    return ms


def _band_matrices(D):
    """D_k[r,s] = f_k[r-s] for 0<=r-s<KS;  S_k[r,s] = f_k[128+r-s] for s-r>=65."""
    Dm = np.zeros((K, 128, 128), np.float32)
    Sm = np.zeros((K, 128, 128), np.float32)
    r = np.arange(128)[:, None]
    s = np.arange(128)[None, :]
    d1 = r - s
    d2 = 128 + r - s
    m1 = (d1 >= 0) & (d1 < KS)
    m2 = (d2 > 0) & (d2 < KS)
    for k in range(K):
        Dm[k][m1] = D[k][d1[m1]]
        Sm[k][m2] = D[k][d2[m2]]
    return Dm, Sm


def _dedup_ldweights(d):
    """Remove Ldweights whose weight AP is identical to the previous PE
    weight load with only Matmults in between — the stationary operand is
    still in the array.  (bass emits one Ldweights per matmul, even for
    back-to-back matmuls sharing lhsT.)  Any waits on a removed Ldweights
    move onto the next PE instruction (the legalizer splits them later)."""
    for fn in d["functions"]:
        for bb in fn["blocks"]:
            out = []
            prev_key = None
            pending_waits = []
            for inst in bb["instructions"]:
                op = inst["opcode"]
                if op == "Ldweights":
                    w = inst["ins"][0]
                    key = (w.get("memref"), w.get("offset"), str(w.get("ap")),
                           str(inst.get("tile_position")))
                    si = inst.get("sync_info")
                    if key == prev_key:
                        if si and si.get("on_wait"):
                            pending_waits.extend(si["on_wait"])
                        assert not (si and si.get("on_update"))
                        continue
                    prev_key = key
                elif op == "Matmult":
                    if pending_waits:
                        si = inst.get("sync_info")
                        if si is None:
                            si = {"on_wait": [], "on_update": []}
                            inst["sync_info"] = si
                        si["on_wait"] = list(si.get("on_wait", [])) + pending_waits
                        pending_waits = []
                elif inst.get("engine") == "PE":
                    prev_key = None
                    if pending_waits:
                        si = inst.get("sync_info")
                        if si is None:
                            si = {"on_wait": [], "on_update": []}
                            inst["sync_info"] = si
                        si["on_wait"] = list(si.get("on_wait", [])) + pending_waits
                        pending_waits = []
                out.append(inst)
            assert not pending_waits
            bb["instructions"] = out
    return d


def _legalize_bir(bir_bytes):
    """The walrus build here encodes at most ONE sync-wait per instruction
    ("Too many sync wait commands").  Tile attaches up to 3.  Split the
    extras onto EventSemaphore wait-carrier instructions inserted just
    before, on the same engine (engine streams keep BB relative order, so
    the carriers execute immediately before the original)."""
    import orjson

    d = orjson.loads(bir_bytes)
    _dedup_ldweights(d)
    for fn in d["functions"]:
        for bb in fn["blocks"]:
            out = []
            for inst in bb["instructions"]:
                si = inst.get("sync_info")
                ow = si.get("on_wait", []) if si else []
                if len(ow) > 1:
                    for j, w in enumerate(ow[:-1]):
                        out.append({
                            "debug": inst.get("debug", 0),
                            "engine": inst["engine"],
                            "ins": [],
                            "outs": [],
                            "name": f"{inst['name']}_wsplit{j}",
                            "opcode": "EventSemaphore",
                            "sync_info": {"on_update": [], "on_wait": [w]},
                        })
                    si["on_wait"] = [ow[-1]]
                out.append(inst)
            bb["instructions"] = out
    return orjson.dumps(d)


def _install_patches():
    import concourse.bass2jax as b2j
    from concourse.bass_utils import compile_bir_kernel as _cbk

    def _cbk_legal(bir_str, compile_dir_path, neff_name):
        return _cbk(_legalize_bir(bir_str), compile_dir_path,
                    neff_name=neff_name)

    b2j.compile_bir_kernel = _cbk_legal


def _build_program():
    import concourse.bass as bass
    import concourse.mybir as mybir
    import concourse.tile as tile
    import bass_rust
    from concourse.tile import add_dep_helper as add_dep
    from concourse.vector_clock import ScopedClock

    _install_patches()

    # The nix walrus build rejects >1 sync-wait on CTRL-class (Drain)
    # instructions; split the Tile tail-drain waits across a chain of
    # single-wait drains.
    def _drain_and_barrier(self, tick_clock, wait_clock):
        drain_inst = self.nc.sync.drain()
        wait_clock.add_sem_waits(
            drain_inst.ins, ScopedClock({None: tick_clock.global_clock})
        )
        si = drain_inst.ins.sync_info
        waits = list(si.on_wait) if si is not None else []
        if len(waits) > 1:
            si.on_wait = waits[:1]
            for w in waits[1:]:
                d = self.nc.sync.drain()
                d.ins.sync_info = bass_rust.SyncInfo(on_wait=[w], on_update=[])
        self.nc.all_engine_barrier()
        assert self.sems is not None
        popped = self.nc._tile_sem_poison_stack.pop()
        assert popped is self._sem_poison
        self.nc.clear_and_free_semaphores(list(self.sems.allocated().values()))
        self.nc.all_engine_barrier()

    tile.TileContext._drain_and_barrier = _drain_and_barrier

    f32 = mybir.dt.float32
    f16 = mybir.dt.float16
    Alu = mybir.AluOpType
    ms = _momentum_coeffs()

    nc = bass.Bass("TRN2", target_bir_lowering=False, debug=False,
                   num_devices=NCORES)
    d_sig = nc.dram_tensor("sig", [128, CW], f32, kind="ExternalInput").ap()
    d_w1 = nc.dram_tensor("w1", [128, 2 * K * 128], f16, kind="ExternalInput").ap()
    d_w2 = nc.dram_tensor("w2", [128, 2 * K * 128], f16, kind="ExternalInput").ap()
    d_wid = nc.dram_tensor("wid", [128, 2 * T * 128], f16, kind="ExternalInput").ap()
    d_out = nc.dram_tensor("xout", [128, K * CW], f32, kind="ExternalOutput").ap()

    with tile.TileContext(nc) as tc:
        with (
            tc.tile_pool(name="const", bufs=1) as const,
            tc.tile_pool(name="state", bufs=1) as state,
            tc.tile_pool(name="psq", bufs=2, space="PSUM") as psqp,
            tc.tile_pool(name="psu", bufs=2, space="PSUM") as psup,
            tc.tile_pool(name="psj", bufs=1, space="PSUM") as psjp,
        ):
            w1 = const.tile([128, 2 * K * 128], f16)
            w2 = const.tile([128, 2 * K * 128], f16)
            wid = const.tile([128, 2 * T * 128], f16)
            sigt = const.tile([128, CW], f32)
            nc.sync.dma_start(sigt[:], d_sig[:])
            nc.sync.dma_start(w2[:], d_w2[:])   # t=0 needs w2 first
            nc.sync.dma_start(w1[:], d_w1[:])
            nc.sync.dma_start(wid[:], d_wid[:])

            # All iteration state is split into two independent half-tensors
            # (k 0-3 / k 4-7, one PSUM bank each) so Tile's per-tile
            # dependency tracking lets iteration t+1's conv1 start as soon
            # as half 0 has been shrunk+cast, while half 1 is still in
            # flight.
            KH = K // 2               # 4 filters per half
            HB = KH * CW              # 512 columns per half
            X32h = [state.tile([128, HB], f32, name=f"X32h{h}")
                    for h in range(2)]
            Xbh = [[state.tile([128, KH * CWH], f16, name=f"Xb{a}{h}")
                    for h in range(2)]
                   for a in range(2)]             # [buf A/B][half]
            v16 = state.tile([128, CWH], f16)
            btA = state.tile([128, CW], f32)
            btB = state.tile([128, CW], f32)
            u_h = [state.tile([128, HB], f32, name=f"u_h{h}")
                   for h in range(2)]
            tclh = [state.tile([128, HB], f32, name=f"tclh{h}")
                    for h in range(2)]

            for h in range(2):
                nc.gpsimd.memset(X32h[h][:], 0.0)
                nc.gpsimd.memset(Xbh[0][h][:], 0.0)
                nc.gpsimd.memset(Xbh[1][h][:], 0.0)

            X32h3 = [x.rearrange("p (k c) -> p k c", k=KH) for x in X32h]

            def body(thr_f: float):
                for t in range(T):
                    m = float(ms[t])
                    Xc, Xp = (Xbh[0], Xbh[1]) if t % 2 == 0 else (Xbh[1], Xbh[0])
                    # btmp_t = m_t * r_{t-1} + sig; r_0 = 0 so t<=1 uses sig
                    bt_cur = sigt if t <= 1 else (btA if t % 2 == 0 else btB)
                    bt_next = btB if t % 2 == 0 else btA

                    psu = [psup.tile([128, HB], f32, tag=f"psu{h}",
                                     name=f"psu{h}_{t}")
                           for h in range(2)]

                    if t > 0:
                        # conv1: q = H @ x_t (accumulate over k, 2 bands).
                        # Emit k 0-3 (half 0), then half 0's momentum
                        # matmuls, then k 4-7 — half 1's fp16 copy lands
                        # later than half 0's, so give the PE half-0-only
                        # work to chew on in between.
                        psq = psqp.tile([128, CW], f32)
                        for k in range(K):
                            if k == KH and t > 0:
                                for km in range(KH):
                                    nc.tensor.matmul(
                                        psu[0][:, km * CW:(km + 1) * CW],
                                        wid[:, (2 * t) * 128:(2 * t + 1) * 128],
                                        Xc[0][:, km * CWH + BL:(km + 1) * CWH],
                                        start=(km == 0), stop=False)
                                for km in range(KH):
                                    nc.tensor.matmul(
                                        psu[0][:, km * CW:(km + 1) * CW],
                                        wid[:, (2 * t + 1) * 128:(2 * t + 2) * 128],
                                        Xp[0][:, km * CWH + BL:(km + 1) * CWH],
                                        start=False, stop=False)
                            Xs = Xc[k // KH]
                            o = (k % KH) * CWH
                            nc.tensor.matmul(
                                psq[:],
                                w1[:, (2 * k) * 128:(2 * k + 1) * 128],
                                Xs[:, o + BL: o + CWH],
                                start=(k == 0), stop=False,
                            )
                            nc.tensor.matmul(
                                psq[:],
                                w1[:, (2 * k + 1) * 128:(2 * k + 2) * 128],
                                Xs[:, o: o + CW],
                                start=False, stop=(k == K - 1),
                            )

                        # v = -(1+m) q + btmp (written directly as fp16);
                        # btmp_{t+1} = m_{t+1} q + sig
                        nc.vector.scalar_tensor_tensor(
                            v16[:, 0:CW], psq[:], -(1.0 + m), bt_cur[:],
                            Alu.mult, Alu.add)
                        nc.vector.tensor_copy(v16[:, CW:CWH], v16[:, 0:BL])
                        if t + 1 < T:
                            nc.vector.scalar_tensor_tensor(
                                bt_next[:], psq[:], float(ms[t + 1]), sigt[:],
                                Alu.mult, Alu.add)
                    else:
                        # t=0: x=0 -> q=0, v = sig; momentum terms all zero
                        nc.scalar.copy(v16[:, 0:CW], sigt[:])
                        nc.vector.tensor_copy(v16[:, CW:CWH], v16[:, 0:BL])

                    # Per half: momentum passthrough (no v dependency — runs
                    # while DVE does the v-chain), then conv2 bands, then the
                    # shrink chain.  Emitting h=0's PSUM work before h=1's
                    # lets the h=0 shrink overlap h=1's matmuls.
                    h0_tail = None
                    ts_h = [None, None]
                    tt_h = [None, None]
                    for h in range(2):
                        if t > 0 and h == 1:
                            for k in range(KH):
                                nc.tensor.matmul(
                                    psu[h][:, k * CW:(k + 1) * CW],
                                    wid[:, (2 * t) * 128:(2 * t + 1) * 128],
                                    Xc[h][:, k * CWH + BL:(k + 1) * CWH],
                                    start=(k == 0), stop=False)
                            for k in range(KH):
                                nc.tensor.matmul(
                                    psu[h][:, k * CW:(k + 1) * CW],
                                    wid[:, (2 * t + 1) * 128:(2 * t + 2) * 128],
                                    Xp[h][:, k * CWH + BL:(k + 1) * CWH],
                                    start=False, stop=False)
                        for k in range(KH):
                            kg = h * KH + k
                            reg = psu[h][:, k * CW:(k + 1) * CW]
                            nc.tensor.matmul(
                                reg, w2[:, (2 * kg) * 128:(2 * kg + 1) * 128],
                                v16[:, 0:CW],
                                start=(t == 0 and k == 0), stop=False)
                            nc.tensor.matmul(
                                reg, w2[:, (2 * kg + 1) * 128:(2 * kg + 2) * 128],
                                v16[:, BL:CWH], start=False,
                                stop=(k == KH - 1))

                        # softshrink x_{t+1} = u - clamp(u,-thr,thr), u=x+psu
                        Xp3 = Xp[h].rearrange("p (k c) -> p k c", k=KH)
                        i_stt = nc.vector.scalar_tensor_tensor(
                            u_h[h][:], psu[h][:], 1.0, X32h[h][:],
                            Alu.mult, Alu.add)
                        del i_stt  # scheduler interleaving is net-positive
                        i_ts = nc.vector.tensor_scalar(
                            tclh[h][:], u_h[h][:], -thr_f, thr_f,
                            Alu.max, Alu.min)
                        ts_h[h] = i_ts
                        i_tt = nc.vector.tensor_sub(
                            X32h[h][:], u_h[h][:], tclh[h][:])
                        tt_h[h] = i_tt
                        # fp16 copies on DVE right behind the sub — no
                        # cross-engine hop on the path to t+1's conv1
                        nc.vector.tensor_copy(
                            Xp3[:, :, BL:CWH], X32h3[h][:])
                        nc.vector.tensor_copy(
                            Xp3[:, :, 0:BL],
                            X32h3[h][:, :, CW - BL:CW])

                    if t > 0:
                        # HAM keep-warm: throwaway matmuls pinned to run
                        # mid-shrink (after each half's clamp) so the PE
                        # never sees a full 3.4us idle window and stays at
                        # 2.4GHz.
                        for j, anchor in enumerate((ts_h[0], ts_h[1])):
                            junk = psjp.tile([128, CW], f32,
                                             name=f"junk_{t}_{j}", tag="junk")
                            i_j = nc.tensor.matmul(
                                junk[:], w2[:, 0:128], v16[:, 0:CW],
                                start=True, stop=True)
                            add_dep(i_j.ins, anchor.ins, sync=True,
                                    reason="HAM keep-warm mid-gap")

            body(_CACHE["thr"])
            nc.sync.dma_start(d_out[:, 0:HB], X32h[0][:])
            nc.sync.dma_start(d_out[:, HB:2 * HB], X32h[1][:])

    return nc


def kernel(signal, local_dictionary):
    sig = np.ascontiguousarray(np.asarray(signal, dtype=np.float32))
    D = np.ascontiguousarray(np.asarray(local_dictionary, dtype=np.float32))
    assert sig.shape == (N, B) and D.shape == (K, KS)

    # Lipschitz constant: H H^T = F^H diag(sum_k |fft(f_k)|^2) F  (circulants)
    fpad = np.zeros((K, N), np.float64)
    fpad[:, :KS] = D.astype(np.float64)
    L = np.float32((np.abs(np.fft.fft(fpad, axis=1)) ** 2).sum(0).max() + 1.0)
    thr = np.float32(LAM / L)
    _CACHE["thr"] = float(thr)

    Dm, Sm = _band_matrices(D)
    ms = _momentum_coeffs()

    # conv1 lhsT[j,i] = D_k[i,j]  (transposed);  conv2 lhsT[i,j] = D_k[i,j]/L
    w1 = np.empty((128, 2 * K * 128), np.float16)
    w2 = np.empty((128, 2 * K * 128), np.float16)
    for k in range(K):
        w1[:, (2 * k) * 128:(2 * k + 1) * 128] = Dm[k].T.astype(np.float16)
        w1[:, (2 * k + 1) * 128:(2 * k + 2) * 128] = Sm[k].T.astype(np.float16)
        w2[:, (2 * k) * 128:(2 * k + 1) * 128] = (Dm[k] / L).astype(np.float16)
        w2[:, (2 * k + 1) * 128:(2 * k + 2) * 128] = (Sm[k] / L).astype(np.float16)
    eye = np.eye(128, dtype=np.float32)
    wid = np.empty((128, 2 * T * 128), np.float16)
    for t in range(T):
        wid[:, (2 * t) * 128:(2 * t + 1) * 128] = (ms[t] * eye).astype(np.float16)
        wid[:, (2 * t + 1) * 128:(2 * t + 2) * 128] = (-ms[t] * eye).astype(np.float16)

    nc = _build_program()

    from concourse.bass_utils import run_bass_kernel_spmd

    in_maps = []
    for c in range(NCORES):
        sc = sig[:, c * BL:(c + 1) * BL]                      # [2048, 8]
        sc = sc.reshape(NB, 128, BL).transpose(1, 0, 2).reshape(128, CW)
        in_maps.append({
            "sig": np.ascontiguousarray(sc),
            "w1": w1, "w2": w2, "wid": wid,
        })

    _CACHE["in_maps"] = in_maps
    res = run_bass_kernel_spmd(nc, in_maps, list(range(NCORES)))

    out = np.empty((K * N, B), np.float32)
    for c in range(NCORES):
        xc = res.results[c]["xout"]                           # [128, 1024]
        xc = xc.reshape(128, K, NB, BL).transpose(1, 2, 0, 3).reshape(K * N, BL)
        out[:, c * BL:(c + 1) * BL] = xc
    return out


# revision 40
# speedup vs baseline: 1.0917x; 1.0874x over previous
"""CRsAE1d FISTA kernel for 8 Trainium2 NeuronCores.

Strategy
--------
H = [circ(f_0)|...|circ(f_7)] is block-circulant: with 128-row blocking each
circulant is block-bidiagonal with ONE repeated diagonal block D_k (lower-band
Toeplitz, f[0..63]) and ONE repeated subdiagonal block S_k (upper-corner band,
f[1..63]).  So H@w and H^T@v are 16 tiny [128,128] matmuls each (per filter:
one D-band + one S-band matmul whose rhs is a block-shifted view of the
operand, with circular wrap handled by an 8-column halo copy).

Data-parallel over batch: 64 columns -> 8 cores x 8 columns.  Everything lives
in SBUF; per iteration the PE does 48 fp16 matmuls (conv1 16, conv2 16,
momentum-passthrough 16 via +/- m_t scaled-identity weights accumulated into
the same PSUM region), DVE does the v-combine and softshrink
(x - clamp(x, -thr, thr)) with the f32 master copy of x kept exactly.

fp16 weights/activations with an f32 master for x gives ~4e-4 rel err vs the
f32 reference (measured in numpy bit-sim); bf16 would give ~3e-3.
"""

import sys

for p in ("/root/.axon_site", "/root/.axon_site/_ro/trn_rl_repo",
          "/root/.axon_site/_ro/pypackages", "/opt/trn_rl_repo"):
    if p not in sys.path:
        sys.path.append(p)

import numpy as np

T = 15
LAM = 0.1
N = 2048
K = 8
KS = 64
B = 64
NCORES = 8
BL = B // NCORES          # batch per core
NB = N // 128             # 16 row-blocks
CW = NB * BL              # 128 columns per (k) region in (J,b) layout
CWH = CW + BL             # body + 8-column halo

_CACHE: dict = {}


def _momentum_coeffs():
    s = 0.0
    ms = []
    for _ in range(T):
        st = (1.0 + np.sqrt(1.0 + 4.0 * s * s)) / 2.0
        ms.append(np.float32((s - 1.0) / st))
        s = st
    return ms


def _band_matrices(D):
    """D_k[r,s] = f_k[r-s] for 0<=r-s<KS;  S_k[r,s] = f_k[128+r-s] for s-r>=65."""
    Dm = np.zeros((K, 128, 128), np.float32)
    Sm = np.zeros((K, 128, 128), np.float32)
    r = np.arange(128)[:, None]
    s = np.arange(128)[None, :]
    d1 = r - s
    d2 = 128 + r - s
    m1 = (d1 >= 0) & (d1 < KS)
    m2 = (d2 > 0) & (d2 < KS)
    for k in range(K):
        Dm[k][m1] = D[k][d1[m1]]
        Sm[k][m2] = D[k][d2[m2]]
    return Dm, Sm


def _dedup_ldweights(d):
    """Remove Ldweights whose weight AP is identical to the previous PE
    weight load with only Matmults in between — the stationary operand is
    still in the array.  (bass emits one Ldweights per matmul, even for
    back-to-back matmuls sharing lhsT.)  Any waits on a removed Ldweights
    move onto the next PE instruction (the legalizer splits them later)."""
    for fn in d["functions"]:
        for bb in fn["blocks"]:
            out = []
            prev_key = None
            pending_waits = []
            for inst in bb["instructions"]:
                op = inst["opcode"]
                if op == "Ldweights":
                    w = inst["ins"][0]
                    key = (w.get("memref"), w.get("offset"), str(w.get("ap")),
                           str(inst.get("tile_position")))
                    si = inst.get("sync_info")
                    if key == prev_key:
                        if si and si.get("on_wait"):
                            pending_waits.extend(si["on_wait"])
                        assert not (si and si.get("on_update"))
                        continue
                    prev_key = key
                elif op == "Matmult":
                    if pending_waits:
                        si = inst.get("sync_info")
                        if si is None:
                            si = {"on_wait": [], "on_update": []}
                            inst["sync_info"] = si
                        si["on_wait"] = list(si.get("on_wait", [])) + pending_waits
                        pending_waits = []
                elif inst.get("engine") == "PE":
                    prev_key = None
                    if pending_waits:
                        si = inst.get("sync_info")
                        if si is None:
                            si = {"on_wait": [], "on_update": []}
                            inst["sync_info"] = si
                        si["on_wait"] = list(si.get("on_wait", [])) + pending_waits
                        pending_waits = []
                out.append(inst)
            assert not pending_waits
            bb["instructions"] = out
    return d


def _legalize_bir(bir_bytes):
    """The walrus build here encodes at most ONE sync-wait per instruction
    ("Too many sync wait commands").  Tile attaches up to 3.  Split the
    extras onto EventSemaphore wait-carrier instructions inserted just
    before, on the same engine (engine streams keep BB relative order, so
    the carriers execute immediately before the original)."""
    import orjson

    d = orjson.loads(bir_bytes)
    _dedup_ldweights(d)
    for fn in d["functions"]:
        for bb in fn["blocks"]:
            out = []
            for inst in bb["instructions"]:
                si = inst.get("sync_info")
                ow = si.get("on_wait", []) if si else []
                if len(ow) > 1:
                    for j, w in enumerate(ow[:-1]):
                        out.append({
                            "debug": inst.get("debug", 0),
                            "engine": inst["engine"],
                            "ins": [],
                            "outs": [],
                            "name": f"{inst['name']}_wsplit{j}",
                            "opcode": "EventSemaphore",
                            "sync_info": {"on_update": [], "on_wait": [w]},
                        })
                    si["on_wait"] = [ow[-1]]
                out.append(inst)
            bb["instructions"] = out
    return orjson.dumps(d)


def _install_patches():
    import concourse.bass2jax as b2j
    from concourse.bass_utils import compile_bir_kernel as _cbk

    def _cbk_legal(bir_str, compile_dir_path, neff_name):
        return _cbk(_legalize_bir(bir_str), compile_dir_path,
                    neff_name=neff_name)

    b2j.compile_bir_kernel = _cbk_legal


def _build_program():
    import concourse.bass as bass
    import concourse.mybir as mybir
    import concourse.tile as tile
    import bass_rust
    from concourse.tile import add_dep_helper as add_dep
    from concourse.vector_clock import ScopedClock

    _install_patches()

    # The nix walrus build rejects >1 sync-wait on CTRL-class (Drain)
    # instructions; split the Tile tail-drain waits across a chain of
    # single-wait drains.
    def _drain_and_barrier(self, tick_clock, wait_clock):
        drain_inst = self.nc.sync.drain()
        wait_clock.add_sem_waits(
            drain_inst.ins, ScopedClock({None: tick_clock.global_clock})
        )
        si = drain_inst.ins.sync_info
        waits = list(si.on_wait) if si is not None else []
        if len(waits) > 1:
            si.on_wait = waits[:1]
            for w in waits[1:]:
                d = self.nc.sync.drain()
                d.ins.sync_info = bass_rust.SyncInfo(on_wait=[w], on_update=[])
        self.nc.all_engine_barrier()
        assert self.sems is not None
        popped = self.nc._tile_sem_poison_stack.pop()
        assert popped is self._sem_poison
        self.nc.clear_and_free_semaphores(list(self.sems.allocated().values()))
        self.nc.all_engine_barrier()

    tile.TileContext._drain_and_barrier = _drain_and_barrier

    f32 = mybir.dt.float32
    f16 = mybir.dt.float16
    Alu = mybir.AluOpType
    ms = _momentum_coeffs()

    nc = bass.Bass("TRN2", target_bir_lowering=False, debug=False,
                   num_devices=NCORES)
    d_sig = nc.dram_tensor("sig", [128, CW], f32, kind="ExternalInput").ap()
    d_w1 = nc.dram_tensor("w1", [128, 2 * K * 128], f16, kind="ExternalInput").ap()
    d_w2 = nc.dram_tensor("w2", [128, 2 * K * 128], f16, kind="ExternalInput").ap()
    d_wid = nc.dram_tensor("wid", [128, 2 * T * 128], f16, kind="ExternalInput").ap()
    d_out = nc.dram_tensor("xout", [128, K * CW], f32, kind="ExternalOutput").ap()

    with tile.TileContext(nc) as tc:
        with (
            tc.tile_pool(name="const", bufs=1) as const,
            tc.tile_pool(name="state", bufs=1) as state,
            tc.tile_pool(name="psq", bufs=2, space="PSUM") as psqp,
            tc.tile_pool(name="psu", bufs=2, space="PSUM") as psup,
            tc.tile_pool(name="psj", bufs=1, space="PSUM") as psjp,
        ):
            w1 = const.tile([128, 2 * K * 128], f16)
            w2 = const.tile([128, 2 * K * 128], f16)
            wid = const.tile([128, 2 * T * 128], f16)
            sigt = const.tile([128, CW], f32)
            nc.sync.dma_start(sigt[:], d_sig[:])
            nc.sync.dma_start(w2[:], d_w2[:])   # t=0 needs w2 first
            nc.sync.dma_start(w1[:], d_w1[:])
            nc.sync.dma_start(wid[:], d_wid[:])

            # All iteration state is split into two independent half-tensors
            # (k 0-3 / k 4-7, one PSUM bank each) so Tile's per-tile
            # dependency tracking lets iteration t+1's conv1 start as soon
            # as half 0 has been shrunk+cast, while half 1 is still in
            # flight.
            KH = K // 2               # 4 filters per half
            HB = KH * CW              # 512 columns per half
            X32h = [state.tile([128, HB], f32, name=f"X32h{h}")
                    for h in range(2)]
            Xbh = [[state.tile([128, KH * CWH], f16, name=f"Xb{a}{h}")
                    for h in range(2)]
                   for a in range(2)]             # [buf A/B][half]
            v16 = state.tile([128, CWH], f16)
            btA = state.tile([128, CW], f32)
            btB = state.tile([128, CW], f32)
            u_h = [state.tile([128, HB], f32, name=f"u_h{h}")
                   for h in range(2)]
            tclh = [state.tile([128, HB], f32, name=f"tclh{h}")
                    for h in range(2)]

            for h in range(2):
                nc.gpsimd.memset(X32h[h][:], 0.0)
                nc.gpsimd.memset(Xbh[0][h][:], 0.0)
                nc.gpsimd.memset(Xbh[1][h][:], 0.0)

            X32h3 = [x.rearrange("p (k c) -> p k c", k=KH) for x in X32h]

            def body(thr_f: float):
                for t in range(T):
                    m = float(ms[t])
                    Xc, Xp = (Xbh[0], Xbh[1]) if t % 2 == 0 else (Xbh[1], Xbh[0])
                    # btmp_t = m_t * r_{t-1} + sig; r_0 = 0 so t<=1 uses sig
                    bt_cur = sigt if t <= 1 else (btA if t % 2 == 0 else btB)
                    bt_next = btB if t % 2 == 0 else btA

                    psu = [psup.tile([128, HB], f32, tag=f"psu{h}",
                                     name=f"psu{h}_{t}")
                           for h in range(2)]

                    if t > 0:
                        # conv1: q = H @ x_t (accumulate over k, 2 bands).
                        # Emit k 0-3 (half 0), then half 0's momentum
                        # matmuls, then k 4-7 — half 1's fp16 copy lands
                        # later than half 0's, so give the PE half-0-only
                        # work to chew on in between.
                        psq = psqp.tile([128, CW], f32)
                        for k in range(K):
                            if k == KH and t > 0:
                                for km in range(KH):
                                    nc.tensor.matmul(
                                        psu[0][:, km * CW:(km + 1) * CW],
                                        wid[:, (2 * t) * 128:(2 * t + 1) * 128],
                                        Xc[0][:, km * CWH + BL:(km + 1) * CWH],
                                        start=(km == 0), stop=False)
                                for km in range(KH):
                                    nc.tensor.matmul(
                                        psu[0][:, km * CW:(km + 1) * CW],
                                        wid[:, (2 * t + 1) * 128:(2 * t + 2) * 128],
                                        Xp[0][:, km * CWH + BL:(km + 1) * CWH],
                                        start=False, stop=False)
                            Xs = Xc[k // KH]
                            o = (k % KH) * CWH
                            nc.tensor.matmul(
                                psq[:],
                                w1[:, (2 * k) * 128:(2 * k + 1) * 128],
                                Xs[:, o + BL: o + CWH],
                                start=(k == 0), stop=False,
                            )
                            nc.tensor.matmul(
                                psq[:],
                                w1[:, (2 * k + 1) * 128:(2 * k + 2) * 128],
                                Xs[:, o: o + CW],
                                start=False, stop=(k == K - 1),
                            )

                        # v = -(1+m) q + btmp (written directly as fp16);
                        # btmp_{t+1} = m_{t+1} q + sig
                        nc.vector.scalar_tensor_tensor(
                            v16[:, 0:CW], psq[:], -(1.0 + m), bt_cur[:],
                            Alu.mult, Alu.add)
                        nc.vector.tensor_copy(v16[:, CW:CWH], v16[:, 0:BL])
                    else:
                        # t=0: x=0 -> q=0, v = sig; momentum terms all zero
                        nc.scalar.copy(v16[:, 0:CW], sigt[:])
                        nc.vector.tensor_copy(v16[:, CW:CWH], v16[:, 0:BL])

                    # Per half: momentum passthrough (no v dependency — runs
                    # while DVE does the v-chain), then conv2 bands, then the
                    # shrink chain.  Emitting h=0's PSUM work before h=1's
                    # lets the h=0 shrink overlap h=1's matmuls.
                    h0_tail = None
                    ts_h = [None, None]
                    tt_h = [None, None]
                    for h in range(2):
                        if t > 0 and h == 1:
                            for k in range(KH):
                                nc.tensor.matmul(
                                    psu[h][:, k * CW:(k + 1) * CW],
                                    wid[:, (2 * t) * 128:(2 * t + 1) * 128],
                                    Xc[h][:, k * CWH + BL:(k + 1) * CWH],
                                    start=(k == 0), stop=False)
                            for k in range(KH):
                                nc.tensor.matmul(
                                    psu[h][:, k * CW:(k + 1) * CW],
                                    wid[:, (2 * t + 1) * 128:(2 * t + 2) * 128],
                                    Xp[h][:, k * CWH + BL:(k + 1) * CWH],
                                    start=False, stop=False)
                        for k in range(KH):
                            kg = h * KH + k
                            reg = psu[h][:, k * CW:(k + 1) * CW]
                            nc.tensor.matmul(
                                reg, w2[:, (2 * kg) * 128:(2 * kg + 1) * 128],
                                v16[:, 0:CW],
                                start=(t == 0 and k == 0), stop=False)
                            nc.tensor.matmul(
                                reg, w2[:, (2 * kg + 1) * 128:(2 * kg + 2) * 128],
                                v16[:, BL:CWH], start=False,
                                stop=(k == KH - 1))

                        # softshrink x_{t+1} = u - clamp(u,-thr,thr), u=x+psu
                        Xp3 = Xp[h].rearrange("p (k c) -> p k c", k=KH)
                        i_stt = nc.vector.scalar_tensor_tensor(
                            u_h[h][:], psu[h][:], 1.0, X32h[h][:],
                            Alu.mult, Alu.add)
                        del i_stt  # scheduler interleaving is net-positive
                        i_ts = nc.vector.tensor_scalar(
                            tclh[h][:], u_h[h][:], -thr_f, thr_f,
                            Alu.max, Alu.min)
                        ts_h[h] = i_ts
                        i_tt = nc.vector.tensor_sub(
                            X32h[h][:], u_h[h][:], tclh[h][:])
                        tt_h[h] = i_tt
                        # fp16 copies on DVE right behind the sub — no
                        # cross-engine hop on the path to t+1's conv1
                        nc.vector.tensor_copy(
                            Xp3[:, :, BL:CWH], X32h3[h][:])
                        nc.vector.tensor_copy(
                            Xp3[:, :, 0:BL],
                            X32h3[h][:, :, CW - BL:CW])

                    if t > 0 and t + 1 < T:
                        # btmp_{t+1} = m_{t+1} q + sig — only needed next
                        # iteration; emit after the shrink chain so it
                        # doesn't delay it on DVE
                        nc.vector.scalar_tensor_tensor(
                            bt_next[:], psq[:], float(ms[t + 1]), sigt[:],
                            Alu.mult, Alu.add)

                    if t > 0:
                        # HAM keep-warm: throwaway matmuls pinned to run
                        # mid-shrink (after each half's clamp) so the PE
                        # never sees a full 3.4us idle window and stays at
                        # 2.4GHz.
                        for j, anchor in enumerate((ts_h[0], ts_h[1])):
                            junk = psjp.tile([128, CW], f32,
                                             name=f"junk_{t}_{j}", tag="junk")
                            i_j = nc.tensor.matmul(
                                junk[:], w2[:, 0:128], v16[:, 0:CW],
                                start=True, stop=True)
                            add_dep(i_j.ins, anchor.ins, sync=True,
                                    reason="HAM keep-warm mid-gap")

            body(_CACHE["thr"])
            nc.sync.dma_start(d_out[:, 0:HB], X32h[0][:])
            nc.sync.dma_start(d_out[:, HB:2 * HB], X32h[1][:])

    return nc


def kernel(signal, local_dictionary):
    sig = np.ascontiguousarray(np.asarray(signal, dtype=np.float32))
    D = np.ascontiguousarray(np.asarray(local_dictionary, dtype=np.float32))
    assert sig.shape == (N, B) and D.shape == (K, KS)

    # Lipschitz constant: H H^T = F^H diag(sum_k |fft(f_k)|^2) F  (circulants)
    fpad = np.zeros((K, N), np.float64)
    fpad[:, :KS] = D.astype(np.float64)
    L = np.float32((np.abs(np.fft.fft(fpad, axis=1)) ** 2).sum(0).max() + 1.0)
    thr = np.float32(LAM / L)
    _CACHE["thr"] = float(thr)

    Dm, Sm = _band_matrices(D)
    ms = _momentum_coeffs()

    # conv1 lhsT[j,i] = D_k[i,j]  (transposed);  conv2 lhsT[i,j] = D_k[i,j]/L
    w1 = np.empty((128, 2 * K * 128), np.float16)
    w2 = np.empty((128, 2 * K * 128), np.float16)
    for k in range(K):
        w1[:, (2 * k) * 128:(2 * k + 1) * 128] = Dm[k].T.astype(np.float16)
        w1[:, (2 * k + 1) * 128:(2 * k + 2) * 128] = Sm[k].T.astype(np.float16)
        w2[:, (2 * k) * 128:(2 * k + 1) * 128] = (Dm[k] / L).astype(np.float16)
        w2[:, (2 * k + 1) * 128:(2 * k + 2) * 128] = (Sm[k] / L).astype(np.float16)
    eye = np.eye(128, dtype=np.float32)
    wid = np.empty((128, 2 * T * 128), np.float16)
    for t in range(T):
        wid[:, (2 * t) * 128:(2 * t + 1) * 128] = (ms[t] * eye).astype(np.float16)
        wid[:, (2 * t + 1) * 128:(2 * t + 2) * 128] = (-ms[t] * eye).astype(np.float16)

    nc = _build_program()

    from concourse.bass_utils import run_bass_kernel_spmd

    in_maps = []
    for c in range(NCORES):
        sc = sig[:, c * BL:(c + 1) * BL]                      # [2048, 8]
        sc = sc.reshape(NB, 128, BL).transpose(1, 0, 2).reshape(128, CW)
        in_maps.append({
            "sig": np.ascontiguousarray(sc),
            "w1": w1, "w2": w2, "wid": wid,
        })

    _CACHE["in_maps"] = in_maps
    res = run_bass_kernel_spmd(nc, in_maps, list(range(NCORES)))

    out = np.empty((K * N, B), np.float32)
    for c in range(NCORES):
        xc = res.results[c]["xout"]                           # [128, 1024]
        xc = xc.reshape(128, K, NB, BL).transpose(1, 2, 0, 3).reshape(K * N, BL)
        out[:, c * BL:(c + 1) * BL] = xc
    return out
